# revision 3
# baseline (speedup 1.0000x reference)
"""ChebyKAN linear layer on 8 Trainium2 NeuronCores.

Computation: out[b,o] = sum_{i,d} T_d(tanh(x[b,i])) * coef[i,o,d]
  == sum_d T_d(tanh(x)) @ C_d   (8 accumulated 8192x1024x1024 matmuls
     for d=1..8; the d=0 term sum_i C_0[i,o] is a batch-independent row
     added during the PSUM drain)

Strategy:
  - Data-parallel over batch: core c handles rows [c*1024, (c+1)*1024).
  - Host pre-transposes each core's x slice to (in_features, batch) layout so
    the contraction dim (i) lands on SBUF partitions, and repacks the
    coefficients to (d, i, o) bf16.
  - On-chip: ACT computes tanh in fp32, DVE runs the Chebyshev recursion
    T_d = 2 t T_{d-1} - T_{d-2} in fp32 (scalar_tensor_tensor fuses the
    2*t*T_{d-1} product into one op), ACT casts each T_d to bf16, and PE
    accumulates the 8 degree-matmuls (d=1..8) in fp32 PSUM.
  - Per core the 1024-row batch is processed in two 512-column halves; each
    half keeps its full output (4 b-chunks x 2 o-halves) resident in all
    8 PSUM banks while 64 k-blocks accumulate into it.
  - "S-lag" schedule (default): within a half, the matmuls for batch chunks
    bc2/bc3 lag bc0/bc1 by S k-units. Each half therefore ends with a
    bc23-only tail and begins with a bc01-only head, so the PSUM drains of
    one bank group always overlap the other group's matmuls -- including
    across the half/iteration seam. This removes the ~40us of drain
    serialization the naive schedule pays per call.
  - Ldweights dedup: Tile emits one Ldweights per matmul; the second
    (o-half) matmul on the same stationary reloads it redundantly at ~53ns
    of serial PE time each. A post-compile pass removes exact duplicates,
    transferring any semaphore waits/updates onto the next instruction.

Numerics (validated on HW): rel l2 error vs fp32 reference ~2e-3.

Performance measured via on-device For_i loop slope (the axon tunnel's
~80 ms RPC overhead hides the kernel and NTFF profiling is unavailable
through it). Model: 1040 matmuls x ~210-227ns + ~520 ldweights x 53ns
+ seam slop.
"""

import numpy as np
import ml_dtypes

BATCH = 8192
IN_F = 1024
OUT_F = 1024
DEG = 8  # degree; DEG+1 coefficients per (i,o)
N_CORES = 8
B_CORE = BATCH // N_CORES  # 1024
P = 128
HALF = 512  # batch columns processed per PSUM-resident output block
NI = IN_F // P  # 8 contraction tiles
NBC = HALF // P  # 4 b-chunks per half
NOH = OUT_F // 512  # 2 output halves of 512
N_HALF = B_CORE // HALF  # 2
NU = NI * DEG  # 64 k-units per half

_CACHED_NC = {}


def _build_bass(loop_r=None, variant=""):
    """Build the Bass program. loop_r wraps the whole compute in a hardware
    For loop of loop_r iterations (benchmark-only; slope over loop_r gives
    per-iteration HW time since the axon RPC overhead is per-call)."""
    import contextlib

    import concourse.mybir as mybir
    import concourse.tile as tile
    from concourse import bacc

    f32 = mybir.dt.float32
    bf16 = mybir.dt.bfloat16
    mult = mybir.AluOpType.mult
    sub = mybir.AluOpType.subtract
    add = mybir.AluOpType.add
    Tanh = mybir.ActivationFunctionType.Tanh

    import json as _json

    def _dedup_ldweights(b):
        """Remove back-to-back InstLdweights that reload the identical
        stationary operand (the PE array still holds it). Tile emits one
        Ldweights per matmul, so a weight reused by consecutive matmuls is
        loaded twice; each redundant load costs ~53 ns of serial PE time.
        Semaphore waits/updates on a removed duplicate are transferred to
        the next kept instruction in the same queue (executes later in the
        same in-order stream, so ordering is preserved)."""
        n_removed = 0
        for fn in b.m.functions:
            for blk in fn.blocks:
                # per-engine state: blocks interleave all engines'
                # instructions, and only same-engine (PE) instructions can
                # disturb the loaded stationary or receive transferred sync
                last_key = {}
                pend = {}
                keep = []
                for inst in blk.instructions:
                    eng = getattr(inst, "engine", None)
                    if isinstance(inst, mybir.InstLdweights):
                        d = _json.loads(
                            mybir.instruction_to_pretty_json_string(inst)
                        )
                        key = _json.dumps(
                            [
                                d.get("ins"),
                                d.get("perf_mode"),
                                d.get("is_transpose"),
                                d.get("tile_position"),
                                d.get("tile_size"),
                            ],
                            sort_keys=True,
                        )
                        if key == last_key.get(eng):
                            si = inst.sync_info
                            if si is not None and (si.on_wait or si.on_update):
                                pw, pu = pend.setdefault(eng, ([], []))
                                pw.extend(list(si.on_wait or []))
                                pu.extend(list(si.on_update or []))
                            n_removed += 1
                            continue
                        last_key[eng] = key
                    elif isinstance(
                        inst, (mybir.InstMatmult, mybir.InstEventSemaphore)
                    ):
                        pass  # does not disturb loaded weights
                    else:
                        last_key.pop(eng, None)
                    if eng in pend:
                        pw, pu = pend.pop(eng)
                        si = inst.sync_info
                        if si is None:
                            raise RuntimeError(
                                "dedup: next inst has no sync_info to merge"
                            )
                        si.on_wait = list(si.on_wait or []) + pw
                        si.on_update = list(si.on_update or []) + pu
                    keep.append(inst)
                assert not pend, "dedup: dangling sync at block end"
                blk.instructions[:] = keep

    class _Bacc(bacc.Bacc):
        def compile(self):
            super().compile()
            _dedup_ldweights(self)

    nc = _Bacc(name="chebykan")
    xt = nc.dram_tensor("xt", (IN_F, B_CORE), f32, kind="ExternalInput")
    w = nc.dram_tensor("w", (DEG, IN_F, OUT_F), bf16, kind="ExternalInput")
    wb = nc.dram_tensor("wb", (P, OUT_F), bf16, kind="ExternalInput")
    br = nc.dram_tensor("br", (P, OUT_F), f32, kind="ExternalInput")
    out = nc.dram_tensor("out", (B_CORE, OUT_F), f32, kind="ExternalOutput")

    is_s2 = variant == "" or variant.startswith("s2")

    with (
        tile.TileContext(nc) as tc,
        tc.tile_pool(name="wpool", bufs=14) as wpool,
        tc.tile_pool(name="xpool", bufs=8) as xpool,
        tc.tile_pool(name="tanh", bufs=3) as tanpool,
        tc.tile_pool(name="rec", bufs=6) as rpool,
        tc.tile_pool(name="ch", bufs=80 if variant == "pp" else 20) as chpool,
        tc.tile_pool(name="const", bufs=1) as cpool,
        tc.tile_pool(name="outp", bufs=8) as opool,
        tc.tile_pool(name="psum", bufs=1, space="PSUM") as pspool,
    ):
        if is_s2:
            biasrow = cpool.tile([P, OUT_F], f32)
            nc.sync.dma_start(biasrow[:], br[:, :])
            ones = wbias = None
        else:
            ones = cpool.tile([P, P], bf16)
            nc.vector.memset(ones[:], 1.0)
            wbias = cpool.tile([P, OUT_F], bf16)
            nc.sync.dma_start(wbias[:], wb[:, :])
            biasrow = None

        loop_cm = (
            tc.For_i(
                0,
                loop_r,
                1,
                hint_engines=(mybir.EngineType.PE, mybir.EngineType.SP),
            )
            if loop_r is not None
            else contextlib.nullcontext()
        )
        with loop_cm:
            if is_s2:
                S = 6
                if "_" in variant:
                    S = int(variant.split("_")[1])
                _emit_body_s2(nc, tc, xt, w, out, biasrow,
                              wpool, xpool, tanpool, rpool, chpool, opool,
                              pspool, f32, bf16, mult, sub, add, Tanh, S)
            else:
                _emit_body(nc, tc, xt, w, out, ones, wbias,
                           wpool, xpool, tanpool, rpool, chpool, opool,
                           pspool, f32, bf16, mult, sub, Tanh, variant)
    nc.finalize()
    return nc


def _emit_body_s2(nc, tc, xt, w, out, biasrow,
                  wpool, xpool, tanpool, rpool, chpool, opool, pspool,
                  f32, bf16, mult, sub, add, Tanh, S):
    """S-lag schedule: bc2/bc3 matmuls lag bc0/bc1 by S k-units so each
    bank group's PSUM drain overlaps the other group's matmuls, including
    across half and loop-iteration seams. The d=0 bias row is added during
    the drain (DVE tensor_tensor) instead of a ones-matmul."""
    Tanh_ = Tanh
    for h in range(N_HALF):
        ps = [
            [
                pspool.tile(
                    [P, 512], f32, tag=f"ps_{bc}_{oh}", name=f"ps_{bc}_{oh}"
                )
                for oh in range(NOH)
            ]
            for bc in range(NBC)
        ]
        ch_tiles = {}
        wts = {}
        rec_state = {}

        def produce(u):
            """Emit cheby production (DMA/ACT/DVE) for unit u=(i,d)."""
            i, d = u // DEG, u % DEG + 1
            if d == 1:
                xti = xpool.tile([P, HALF], f32, tag="x")
                nc.sync.dma_start(
                    xti[:],
                    xt[i * P : (i + 1) * P, h * HALF : (h + 1) * HALF],
                )
                t = tanpool.tile([P, HALF], f32, tag="t")
                nc.scalar.activation(t[:], xti[:], Tanh_)
                rec_state[i] = (None, t, t)  # tm2, tm1, t
            tm2, tm1, t = rec_state[i]
            chd = chpool.tile([P, HALF], bf16, tag="ch")
            if d == 1:
                nc.scalar.copy(chd[:], t[:])
                cur = t
            else:
                # pr = (T_{d-1} * 2) * t  (one fused DVE op)
                pr = rpool.tile([P, HALF], f32, tag="rec")
                nc.vector.scalar_tensor_tensor(
                    pr[:], tm1[:], 2.0, t[:], mult, mult
                )
                if d == 2:
                    cur = rpool.tile([P, HALF], f32, tag="rec")
                    nc.vector.tensor_scalar_sub(cur[:], pr[:], 1.0)
                    nc.scalar.copy(chd[:], cur[:])
                elif d < DEG:
                    cur = rpool.tile([P, HALF], f32, tag="rec")
                    nc.vector.tensor_tensor(cur[:], pr[:], tm2[:], sub)
                    nc.scalar.copy(chd[:], cur[:])
                else:
                    # final degree: write the bf16 tile directly
                    cur = None
                    nc.vector.tensor_tensor(chd[:], pr[:], tm2[:], sub)
            rec_state[i] = (tm1, cur, t)
            ch_tiles[u] = chd
            wt = wpool.tile([P, OUT_F], bf16, tag="w")
            nc.sync.dma_start(wt[:], w[d - 1, i * P : (i + 1) * P, :])
            wts[u] = wt

        def cell(u, bc):
            lhsT = ch_tiles[u][:, bc * P : (bc + 1) * P]
            wt = wts[u]
            for oh in range(NOH):
                nc.tensor.matmul(
                    ps[bc][oh],
                    lhsT,
                    wt[:, oh * 512 : (oh + 1) * 512],
                    start=(u == 0),
                    stop=(u == NU - 1),
                )

        def drain(bc, oh):
            ot = opool.tile([P, 512], f32, tag="ot")
            nc.vector.tensor_tensor(
                ot[:], ps[bc][oh], biasrow[:, oh * 512 : (oh + 1) * 512], add
            )
            r0 = h * HALF + bc * P
            nc.sync.dma_start(
                out[r0 : r0 + P, oh * 512 : (oh + 1) * 512], ot[:]
            )

        for u in range(NU):
            produce(u)
            cell(u, 0)
            cell(u, 1)
            if u >= S:
                cell(u - S, 2)
                cell(u - S, 3)
        # bc0/bc1 banks are complete: drain them while the bc23 tail runs
        for bc in (0, 1):
            for oh in range(NOH):
                drain(bc, oh)
        for u in range(NU - S, NU):
            cell(u, 2)
            cell(u, 3)
        # bc2/bc3 drains overlap the next half's bc01-only head
        for bc in (2, 3):
            for oh in range(NOH):
                drain(bc, oh)


def _emit_body(nc, tc, xt, w, out, ones, wbias,
               wpool, xpool, tanpool, rpool, chpool, opool, pspool,
               f32, bf16, mult, sub, Tanh, variant=""):
    if variant == "pp":
        _emit_body_pp(nc, tc, xt, w, out, ones, wbias,
                      wpool, xpool, tanpool, rpool, chpool, opool, pspool,
                      f32, bf16, mult, sub, Tanh)
        return
    n_oh = 1 if variant == "halfmm" else NOH
    for h in range(N_HALF):
            ps = [
                [
                    pspool.tile(
                        [P, 512], f32, tag=f"ps_{bc}_{oh}", name=f"ps_{bc}_{oh}"
                    )
                    for oh in range(n_oh)
                ]
                for bc in range(NBC)
            ]
            # Bias k-block: out += ones.T @ W_bias (covers the d=0 term).
            # start=True clears the PSUM banks.
            for bc in range(NBC):
                for oh in range(n_oh):
                    nc.tensor.matmul(
                        ps[bc][oh],
                        ones,
                        wbias[:, oh * 512 : (oh + 1) * 512],
                        start=True,
                        stop=False,
                    )
            deferred = []
            for i in range(NI):
                xti = xpool.tile([P, HALF], f32, tag="x")
                nc.sync.dma_start(
                    xti[:], xt[i * P : (i + 1) * P, h * HALF : (h + 1) * HALF]
                )
                t = tanpool.tile([P, HALF], f32, tag="t")
                nc.scalar.activation(t[:], xti[:], Tanh)

                tm2 = None  # T_{d-2} (fp32); None encodes T_0 == 1
                tm1 = t  # T_{d-1} (fp32)
                ch1 = None
                for d in range(1, DEG + 1):
                    last = d == DEG
                    if variant == "norec" and d > 1:
                        chd = ch1
                    else:
                        chd = chpool.tile([P, HALF], bf16, tag="ch")
                    if d == 1:
                        nc.scalar.copy(chd[:], t[:])
                        ch1 = chd
                        cur = t
                    elif variant == "norec":
                        cur = None
                    else:
                        # pr = (T_{d-1} * 2) * t  (one fused DVE op)
                        pr = rpool.tile([P, HALF], f32, tag="rec")
                        nc.vector.scalar_tensor_tensor(
                            pr[:], tm1[:], 2.0, t[:], mult, mult
                        )
                        if d == 2:
                            # T_2 = pr - 1
                            cur = rpool.tile([P, HALF], f32, tag="rec")
                            nc.vector.tensor_scalar_sub(cur[:], pr[:], 1.0)
                            nc.scalar.copy(chd[:], cur[:])
                        elif not last:
                            cur = rpool.tile([P, HALF], f32, tag="rec")
                            nc.vector.tensor_tensor(cur[:], pr[:], tm2[:], sub)
                            nc.scalar.copy(chd[:], cur[:])
                        else:
                            # final degree: write the bf16 tile directly
                            cur = None
                            nc.vector.tensor_tensor(chd[:], pr[:], tm2[:], sub)
                    tm2, tm1 = tm1, cur

                    if variant == "nodma":
                        if i == 0 and d == 1:
                            wt0 = wpool.tile([P, 1, OUT_F], bf16, tag="w")
                            nc.sync.dma_start(wt0[:, 0], w[0, 0:P, :])
                        wt = wt0[:, 0]
                    else:
                        wt = wpool.tile([P, OUT_F], bf16, tag="w")
                        nc.sync.dma_start(wt[:], w[d - 1, i * P : (i + 1) * P, :])
                    stop = i == NI - 1 and d == DEG
                    if variant == "stag" and i == NI - 1 and d >= 3:
                        # tail stagger: banks 0-3 finish their k-blocks
                        # before banks 4-7 start theirs, so the 0-3 drains
                        # overlap the 4-7 matmul tail
                        for bc in (0, 1):
                            lhsT = chd[:, bc * P : (bc + 1) * P]
                            for oh in range(n_oh):
                                nc.tensor.matmul(
                                    ps[bc][oh], lhsT,
                                    wt[:, oh * 512 : (oh + 1) * 512],
                                    start=False, stop=stop,
                                )
                        deferred.append((chd, wt, stop))
                        continue
                    for bc in range(NBC):
                        lhsT = chd[:, bc * P : (bc + 1) * P]
                        for oh in range(n_oh):
                            nc.tensor.matmul(
                                ps[bc][oh],
                                lhsT,
                                wt[:, oh * 512 : (oh + 1) * 512],
                                start=False,
                                stop=stop,
                            )
            # deferred bank-4-7 tail (stag variant)
            for chd_, wt_, stop_ in deferred:
                for bc in (2, 3):
                    lhsT = chd_[:, bc * P : (bc + 1) * P]
                    for oh in range(n_oh):
                        nc.tensor.matmul(
                            ps[bc][oh], lhsT,
                            wt_[:, oh * 512 : (oh + 1) * 512],
                            start=False, stop=stop_,
                        )
            # Drain this half's PSUM to SBUF and then HBM. Copies alternate
            # between DVE and ACT to halve the bank-free latency.
            if variant == "nodrain":
                continue
            for bc in range(NBC):
                for oh in range(n_oh):
                    ot = opool.tile([P, 512], f32, tag="ot")
                    if (bc * NOH + oh) % 2 == 0:
                        nc.vector.tensor_copy(ot[:], ps[bc][oh])
                    else:
                        nc.scalar.copy(ot[:], ps[bc][oh])
                    r0 = h * HALF + bc * P
                    nc.sync.dma_start(
                        out[r0 : r0 + P, oh * 512 : (oh + 1) * 512], ot[:]
                    )


def _emit_body_pp(nc, tc, xt, w, out, ones, wbias,
                  wpool, xpool, tanpool, rpool, chpool, opool, pspool,
                  f32, bf16, mult, sub, Tanh):
    """Bank ping-pong: each half runs two passes over all k-blocks, one per
    bank group (bc 0-1 -> banks 0-3, bc 2-3 -> banks 4-7). A group's PSUM
    drain overlaps the other group's matmuls, removing the half-boundary
    serialization. Cheby tiles are computed in pass 0 and reused in pass 1;
    W tiles are re-streamed per pass (2x DMA, still under the PE floor)."""
    for h in range(N_HALF):
        ps = [
            [
                pspool.tile(
                    [P, 512], f32, tag=f"ps_{bc}_{oh}", name=f"ps_{bc}_{oh}"
                )
                for oh in range(NOH)
            ]
            for bc in range(NBC)
        ]
        chs = {}
        for p_ in range(2):
            bcs = (0, 1) if p_ == 0 else (2, 3)
            for bc in bcs:
                for oh in range(NOH):
                    nc.tensor.matmul(
                        ps[bc][oh],
                        ones,
                        wbias[:, oh * 512 : (oh + 1) * 512],
                        start=True,
                        stop=False,
                    )
            for i in range(NI):
                if p_ == 0:
                    xti = xpool.tile([P, HALF], f32, tag="x")
                    nc.sync.dma_start(
                        xti[:],
                        xt[i * P : (i + 1) * P, h * HALF : (h + 1) * HALF],
                    )
                    t = tanpool.tile([P, HALF], f32, tag="t")
                    nc.scalar.activation(t[:], xti[:], Tanh)
                    tm2, tm1 = None, t
                    for d in range(1, DEG + 1):
                        chd = chpool.tile([P, HALF], bf16, tag="ch",
                                          name=f"ch_{h}_{i}_{d}")
                        if d == 1:
                            nc.scalar.copy(chd[:], t[:])
                            cur = t
                        else:
                            pr = rpool.tile([P, HALF], f32, tag="rec")
                            nc.vector.scalar_tensor_tensor(
                                pr[:], tm1[:], 2.0, t[:], mult, mult
                            )
                            if d == 2:
                                cur = rpool.tile([P, HALF], f32, tag="rec")
                                nc.vector.tensor_scalar_sub(cur[:], pr[:], 1.0)
                                nc.scalar.copy(chd[:], cur[:])
                            elif d < DEG:
                                cur = rpool.tile([P, HALF], f32, tag="rec")
                                nc.vector.tensor_tensor(cur[:], pr[:], tm2[:], sub)
                                nc.scalar.copy(chd[:], cur[:])
                            else:
                                cur = None
                                nc.vector.tensor_tensor(chd[:], pr[:], tm2[:], sub)
                        tm2, tm1 = tm1, cur
                        chs[(i, d)] = chd
                for d in range(1, DEG + 1):
                    chd = chs[(i, d)]
                    wt = wpool.tile([P, OUT_F], bf16, tag="w")
                    nc.sync.dma_start(wt[:], w[d - 1, i * P : (i + 1) * P, :])
                    stop = i == NI - 1 and d == DEG
                    for bc in bcs:
                        lhsT = chd[:, bc * P : (bc + 1) * P]
                        for oh in range(NOH):
                            nc.tensor.matmul(
                                ps[bc][oh],
                                lhsT,
                                wt[:, oh * 512 : (oh + 1) * 512],
                                start=False,
                                stop=stop,
                            )
            # drain this bank group; overlaps the other group's compute
            for bc in bcs:
                for oh in range(NOH):
                    ot = opool.tile([P, 512], f32, tag="ot")
                    if (bc * NOH + oh) % 2 == 0:
                        nc.vector.tensor_copy(ot[:], ps[bc][oh])
                    else:
                        nc.scalar.copy(ot[:], ps[bc][oh])
                    r0 = h * HALF + bc * P
                    nc.sync.dma_start(
                        out[r0 : r0 + P, oh * 512 : (oh + 1) * 512], ot[:]
                    )


def _get_nc(loop_r=None, variant=""):
    key = (loop_r, variant)
    if key not in _CACHED_NC:
        _CACHED_NC[key] = _build_bass(loop_r, variant)
    return _CACHED_NC[key]


def _prep_inputs(x, coefficients):
    bf16 = ml_dtypes.bfloat16
    x = np.asarray(x, dtype=np.float32)
    coef = np.asarray(coefficients, dtype=np.float32)
    # (d, i, o) bf16 for d = 1..DEG
    w_all = np.ascontiguousarray(coef.transpose(2, 0, 1)[1 : DEG + 1]).astype(bf16)
    # d=0 term folded over i into a single 128-row contraction block
    # (ones-matmul trick used by the v0/pp/stag variants)
    wb_arr = np.ascontiguousarray(
        coef[:, :, 0].reshape(NI, P, OUT_F).sum(axis=0)
    ).astype(bf16)
    # d=0 term as a full-precision row, replicated across partitions and
    # added during the drain (s2 variants)
    br_row = coef[:, :, 0].sum(axis=0).astype(np.float32)
    br_arr = np.ascontiguousarray(
        np.broadcast_to(br_row[None, :], (P, OUT_F))
    ).astype(np.float32)
    in_maps = []
    for c in range(N_CORES):
        xc = x[c * B_CORE : (c + 1) * B_CORE, :]
        in_maps.append(
            {
                "xt": np.ascontiguousarray(xc.T),
                "w": w_all,
                "wb": wb_arr,
                "br": br_arr,
            }
        )
    return in_maps


def run(x, coefficients, trace=False, tmpdir=None, variant=""):
    """Run on hardware; returns (out, BassKernelResults)."""
    from concourse.bass_utils import run_bass_kernel_spmd

    nc = _get_nc(None, variant)
    in_maps = _prep_inputs(x, coefficients)
    res = run_bass_kernel_spmd(
        nc,
        in_maps,
        core_ids=list(range(N_CORES)),
        trace=trace,
        tmpdir=tmpdir,
    )
    out = np.concatenate([r["out"] for r in res.results], axis=0)
    return np.ascontiguousarray(out, dtype=np.float32), res


def kernel(x, coefficients):
    out, _ = run(x, coefficients, trace=False)
    return out


# revision 11
# speedup vs baseline: 1.0028x; 1.0028x over previous
"""ChebyKAN linear layer on 8 Trainium2 NeuronCores.

Computation: out[b,o] = sum_{i,d} T_d(tanh(x[b,i])) * coef[i,o,d]
  == sum_d T_d(tanh(x)) @ C_d   (8 accumulated 8192x1024x1024 matmuls
     for d=1..8; the d=0 term sum_i C_0[i,o] is a batch-independent row
     added during the PSUM drain)

Strategy:
  - Data-parallel over batch: core c handles rows [c*1024, (c+1)*1024).
  - Host pre-transposes each core's x slice to (in_features, batch) layout so
    the contraction dim (i) lands on SBUF partitions, and repacks the
    coefficients to (d, i, o) bf16.
  - On-chip: ACT computes tanh in fp32, DVE runs the Chebyshev recursion
    T_d = 2 t T_{d-1} - T_{d-2} in fp32 (scalar_tensor_tensor fuses the
    2*t*T_{d-1} product into one op), ACT casts each T_d to bf16, and PE
    accumulates the 8 degree-matmuls (d=1..8) in fp32 PSUM.
  - Per core the 1024-row batch is processed in two 512-column halves; each
    half keeps its full output (4 b-chunks x 2 o-halves) resident in all
    8 PSUM banks while 64 k-blocks accumulate into it.
  - "S-lag" schedule (default): within a half, the matmuls for batch chunks
    bc2/bc3 lag bc0/bc1 by S k-units. Each half therefore ends with a
    bc23-only tail and begins with a bc01-only head, so the PSUM drains of
    one bank group always overlap the other group's matmuls -- including
    across the half/iteration seam. This removes the ~40us of drain
    serialization the naive schedule pays per call.
  - Ldweights dedup: Tile emits one Ldweights per matmul; the second
    (o-half) matmul on the same stationary reloads it redundantly at ~53ns
    of serial PE time each. A post-compile pass removes exact duplicates,
    transferring any semaphore waits/updates onto the next instruction.

Numerics (validated on HW): rel l2 error vs fp32 reference ~2e-3.

Performance measured via on-device For_i loop slope (the axon tunnel's
~80 ms RPC overhead hides the kernel and NTFF profiling is unavailable
through it). Model: 1040 matmuls x ~210-227ns + ~520 ldweights x 53ns
+ seam slop.
"""

import numpy as np
import ml_dtypes

BATCH = 8192
IN_F = 1024
OUT_F = 1024
DEG = 8  # degree; DEG+1 coefficients per (i,o)
N_CORES = 8
B_CORE = BATCH // N_CORES  # 1024
P = 128
HALF = 512  # batch columns processed per PSUM-resident output block
NI = IN_F // P  # 8 contraction tiles
NBC = HALF // P  # 4 b-chunks per half
NOH = OUT_F // 512  # 2 output halves of 512
N_HALF = B_CORE // HALF  # 2
NU = NI * DEG  # 64 k-units per half

_CACHED_NC = {}


def _build_bass(loop_r=None, variant=""):
    """Build the Bass program. loop_r wraps the whole compute in a hardware
    For loop of loop_r iterations (benchmark-only; slope over loop_r gives
    per-iteration HW time since the axon RPC overhead is per-call)."""
    import contextlib

    import concourse.mybir as mybir
    import concourse.tile as tile
    from concourse import bacc

    f32 = mybir.dt.float32
    bf16 = mybir.dt.bfloat16
    mult = mybir.AluOpType.mult
    sub = mybir.AluOpType.subtract
    add = mybir.AluOpType.add
    Tanh = mybir.ActivationFunctionType.Tanh

    import json as _json

    def _dedup_ldweights(b):
        """Remove back-to-back InstLdweights that reload the identical
        stationary operand (the PE array still holds it). Tile emits one
        Ldweights per matmul, so a weight reused by consecutive matmuls is
        loaded twice; each redundant load costs ~53 ns of serial PE time.
        Semaphore waits/updates on a removed duplicate are transferred to
        the next kept instruction in the same queue (executes later in the
        same in-order stream, so ordering is preserved)."""
        n_removed = 0
        for fn in b.m.functions:
            for blk in fn.blocks:
                # per-engine state: blocks interleave all engines'
                # instructions, and only same-engine (PE) instructions can
                # disturb the loaded stationary or receive transferred sync
                last_key = {}
                pend = {}
                keep = []
                for inst in blk.instructions:
                    eng = getattr(inst, "engine", None)
                    if isinstance(inst, mybir.InstLdweights):
                        d = _json.loads(
                            mybir.instruction_to_pretty_json_string(inst)
                        )
                        key = _json.dumps(
                            [
                                d.get("ins"),
                                d.get("perf_mode"),
                                d.get("is_transpose"),
                                d.get("tile_position"),
                                d.get("tile_size"),
                            ],
                            sort_keys=True,
                        )
                        if key == last_key.get(eng):
                            si = inst.sync_info
                            if si is not None and (si.on_wait or si.on_update):
                                pw, pu = pend.setdefault(eng, ([], []))
                                pw.extend(list(si.on_wait or []))
                                pu.extend(list(si.on_update or []))
                            n_removed += 1
                            continue
                        last_key[eng] = key
                    elif isinstance(
                        inst, (mybir.InstMatmult, mybir.InstEventSemaphore)
                    ):
                        pass  # does not disturb loaded weights
                    else:
                        last_key.pop(eng, None)
                    if eng in pend:
                        pw, pu = pend.pop(eng)
                        si = inst.sync_info
                        if si is None:
                            raise RuntimeError(
                                "dedup: next inst has no sync_info to merge"
                            )
                        si.on_wait = list(si.on_wait or []) + pw
                        si.on_update = list(si.on_update or []) + pu
                    keep.append(inst)
                assert not pend, "dedup: dangling sync at block end"
                blk.instructions[:] = keep

    class _Bacc(bacc.Bacc):
        def compile(self):
            super().compile()
            _dedup_ldweights(self)

    nc = _Bacc(name="chebykan")
    xt = nc.dram_tensor("xt", (IN_F, B_CORE), f32, kind="ExternalInput")
    w = nc.dram_tensor("w", (DEG, IN_F, OUT_F), bf16, kind="ExternalInput")
    wb = nc.dram_tensor("wb", (P, OUT_F), bf16, kind="ExternalInput")
    br = nc.dram_tensor("br", (P, OUT_F), f32, kind="ExternalInput")
    out = nc.dram_tensor("out", (B_CORE, OUT_F), f32, kind="ExternalOutput")

    is_s2 = variant == "" or variant.startswith("s2")

    with (
        tile.TileContext(nc) as tc,
        tc.tile_pool(name="wpool", bufs=14) as wpool,
        tc.tile_pool(name="xpool", bufs=8) as xpool,
        tc.tile_pool(name="tanh", bufs=3) as tanpool,
        tc.tile_pool(name="rec", bufs=6) as rpool,
        tc.tile_pool(name="ch", bufs=80 if variant == "pp" else 20) as chpool,
        tc.tile_pool(name="const", bufs=1) as cpool,
        tc.tile_pool(name="outp", bufs=8) as opool,
        tc.tile_pool(name="psum", bufs=1, space="PSUM") as pspool,
    ):
        if is_s2:
            biasrow = cpool.tile([P, OUT_F], f32)
            nc.sync.dma_start(biasrow[:], br[:, :])
            ones = wbias = None
        else:
            ones = cpool.tile([P, P], bf16)
            nc.vector.memset(ones[:], 1.0)
            wbias = cpool.tile([P, OUT_F], bf16)
            nc.sync.dma_start(wbias[:], wb[:, :])
            biasrow = None

        loop_cm = (
            tc.For_i(
                0,
                loop_r,
                1,
                hint_engines=(mybir.EngineType.PE, mybir.EngineType.SP),
            )
            if loop_r is not None
            else contextlib.nullcontext()
        )
        with loop_cm:
            if is_s2:
                S = 6
                if "_" in variant:
                    S = int(variant.split("_")[1])
                _emit_body_s2(nc, tc, xt, w, out, biasrow, ones, wbias,
                              wpool, xpool, tanpool, rpool, chpool, opool,
                              pspool, f32, bf16, mult, sub, add, Tanh, S)
            else:
                _emit_body(nc, tc, xt, w, out, ones, wbias,
                           wpool, xpool, tanpool, rpool, chpool, opool,
                           pspool, f32, bf16, mult, sub, Tanh, variant)
    nc.finalize()
    return nc


def _emit_body_s2(nc, tc, xt, w, out, biasrow, ones, wbias,
                  wpool, xpool, tanpool, rpool, chpool, opool, pspool,
                  f32, bf16, mult, sub, add, Tanh, S):
    """S-lag schedule: bc2/bc3 matmuls lag bc0/bc1 by S k-units so each
    bank group's PSUM drain overlaps the other group's matmuls. Drains are
    DVE tensor_tensor adds that fuse the d=0 bias row; the bc01 drains are
    emitted BEFORE the bc23 tail so the DVE (which runs ahead of the PE and
    has already finished this half's production) executes them during the
    tail instead of head-of-line-blocking the next half's production."""
    Tanh_ = Tanh
    for h in range(N_HALF):
        ps = [
            [
                pspool.tile(
                    [P, 512], f32, tag=f"ps_{bc}_{oh}", name=f"ps_{bc}_{oh}"
                )
                for oh in range(NOH)
            ]
            for bc in range(NBC)
        ]
        ch_tiles = {}
        wts = {}
        rec_state = {}

        def produce(u):
            """Emit cheby production (DMA/ACT/DVE) for unit u=(i,d)."""
            i, d = u // DEG, u % DEG + 1
            if d == 1:
                xti = xpool.tile([P, HALF], f32, tag="x")
                nc.sync.dma_start(
                    xti[:],
                    xt[i * P : (i + 1) * P, h * HALF : (h + 1) * HALF],
                )
                t = tanpool.tile([P, HALF], f32, tag="t")
                nc.scalar.activation(t[:], xti[:], Tanh_)
                rec_state[i] = (None, t, t)  # tm2, tm1, t
            tm2, tm1, t = rec_state[i]
            chd = chpool.tile([P, HALF], bf16, tag="ch")
            if d == 1:
                nc.scalar.copy(chd[:], t[:])
                cur = t
            else:
                # pr = (T_{d-1} * 2) * t  (one fused DVE op)
                pr = rpool.tile([P, HALF], f32, tag="rec")
                nc.vector.scalar_tensor_tensor(
                    pr[:], tm1[:], 2.0, t[:], mult, mult
                )
                if d == 2:
                    cur = rpool.tile([P, HALF], f32, tag="rec")
                    nc.vector.tensor_scalar_sub(cur[:], pr[:], 1.0)
                    nc.scalar.copy(chd[:], cur[:])
                elif d < DEG:
                    cur = rpool.tile([P, HALF], f32, tag="rec")
                    nc.vector.tensor_tensor(cur[:], pr[:], tm2[:], sub)
                    nc.scalar.copy(chd[:], cur[:])
                else:
                    # final degree: write the bf16 tile directly
                    cur = None
                    nc.vector.tensor_tensor(chd[:], pr[:], tm2[:], sub)
            rec_state[i] = (tm1, cur, t)
            ch_tiles[u] = chd
            wt = wpool.tile([P, OUT_F], bf16, tag="w")
            nc.sync.dma_start(wt[:], w[d - 1, i * P : (i + 1) * P, :])
            wts[u] = wt

        def cell(u, bc):
            lhsT = ch_tiles[u][:, bc * P : (bc + 1) * P]
            wt = wts[u]
            for oh in range(NOH):
                nc.tensor.matmul(
                    ps[bc][oh],
                    lhsT,
                    wt[:, oh * 512 : (oh + 1) * 512],
                    start=(u == 0),
                    stop=(u == NU - 1),
                )

        def drain(bc, oh):
            # fused bias add: out_tile = psum + biasrow (the d=0 term)
            ot = opool.tile([P, 512], f32, tag="ot")
            nc.vector.tensor_tensor(
                ot[:], ps[bc][oh],
                biasrow[:, oh * 512 : (oh + 1) * 512], add,
            )
            r0 = h * HALF + bc * P
            nc.sync.dma_start(
                out[r0 : r0 + P, oh * 512 : (oh + 1) * 512], ot[:]
            )

        for u in range(NU):
            produce(u)
            cell(u, 0)
            cell(u, 1)
            if u >= S:
                cell(u - S, 2)
                cell(u - S, 3)
        # bc0/bc1 banks are complete (their stop fired at u=NU-1): drain
        # them now. The DVE reaches these after all production for this
        # half, the waits are already satisfied, and the PE's bc23 tail
        # below covers the copy latency.
        for bc in (0, 1):
            for oh in range(NOH):
                drain(bc, oh)
        for u in range(NU - S, NU):
            cell(u, 2)
            cell(u, 3)
        # bc2/bc3 drains: the next half's bc01-only head covers the bank
        # WAR; the DVE pays a short head-of-line wait here at the seam.
        for bc in (2, 3):
            for oh in range(NOH):
                drain(bc, oh)


def _emit_body(nc, tc, xt, w, out, ones, wbias,
               wpool, xpool, tanpool, rpool, chpool, opool, pspool,
               f32, bf16, mult, sub, Tanh, variant=""):
    if variant == "pp":
        _emit_body_pp(nc, tc, xt, w, out, ones, wbias,
                      wpool, xpool, tanpool, rpool, chpool, opool, pspool,
                      f32, bf16, mult, sub, Tanh)
        return
    n_oh = 1 if variant == "halfmm" else NOH
    for h in range(N_HALF):
            ps = [
                [
                    pspool.tile(
                        [P, 512], f32, tag=f"ps_{bc}_{oh}", name=f"ps_{bc}_{oh}"
                    )
                    for oh in range(n_oh)
                ]
                for bc in range(NBC)
            ]
            # Bias k-block: out += ones.T @ W_bias (covers the d=0 term).
            # start=True clears the PSUM banks.
            for bc in range(NBC):
                for oh in range(n_oh):
                    nc.tensor.matmul(
                        ps[bc][oh],
                        ones,
                        wbias[:, oh * 512 : (oh + 1) * 512],
                        start=True,
                        stop=False,
                    )
            deferred = []
            for i in range(NI):
                xti = xpool.tile([P, HALF], f32, tag="x")
                nc.sync.dma_start(
                    xti[:], xt[i * P : (i + 1) * P, h * HALF : (h + 1) * HALF]
                )
                t = tanpool.tile([P, HALF], f32, tag="t")
                nc.scalar.activation(t[:], xti[:], Tanh)

                tm2 = None  # T_{d-2} (fp32); None encodes T_0 == 1
                tm1 = t  # T_{d-1} (fp32)
                ch1 = None
                for d in range(1, DEG + 1):
                    last = d == DEG
                    if variant == "norec" and d > 1:
                        chd = ch1
                    else:
                        chd = chpool.tile([P, HALF], bf16, tag="ch")
                    if d == 1:
                        nc.scalar.copy(chd[:], t[:])
                        ch1 = chd
                        cur = t
                    elif variant == "norec":
                        cur = None
                    else:
                        # pr = (T_{d-1} * 2) * t  (one fused DVE op)
                        pr = rpool.tile([P, HALF], f32, tag="rec")
                        nc.vector.scalar_tensor_tensor(
                            pr[:], tm1[:], 2.0, t[:], mult, mult
                        )
                        if d == 2:
                            # T_2 = pr - 1
                            cur = rpool.tile([P, HALF], f32, tag="rec")
                            nc.vector.tensor_scalar_sub(cur[:], pr[:], 1.0)
                            nc.scalar.copy(chd[:], cur[:])
                        elif not last:
                            cur = rpool.tile([P, HALF], f32, tag="rec")
                            nc.vector.tensor_tensor(cur[:], pr[:], tm2[:], sub)
                            nc.scalar.copy(chd[:], cur[:])
                        else:
                            # final degree: write the bf16 tile directly
                            cur = None
                            nc.vector.tensor_tensor(chd[:], pr[:], tm2[:], sub)
                    tm2, tm1 = tm1, cur

                    if variant == "nodma":
                        if i == 0 and d == 1:
                            wt0 = wpool.tile([P, 1, OUT_F], bf16, tag="w")
                            nc.sync.dma_start(wt0[:, 0], w[0, 0:P, :])
                        wt = wt0[:, 0]
                    else:
                        wt = wpool.tile([P, OUT_F], bf16, tag="w")
                        nc.sync.dma_start(wt[:], w[d - 1, i * P : (i + 1) * P, :])
                    stop = i == NI - 1 and d == DEG
                    if variant == "stag" and i == NI - 1 and d >= 3:
                        # tail stagger: banks 0-3 finish their k-blocks
                        # before banks 4-7 start theirs, so the 0-3 drains
                        # overlap the 4-7 matmul tail
                        for bc in (0, 1):
                            lhsT = chd[:, bc * P : (bc + 1) * P]
                            for oh in range(n_oh):
                                nc.tensor.matmul(
                                    ps[bc][oh], lhsT,
                                    wt[:, oh * 512 : (oh + 1) * 512],
                                    start=False, stop=stop,
                                )
                        deferred.append((chd, wt, stop))
                        continue
                    for bc in range(NBC):
                        lhsT = chd[:, bc * P : (bc + 1) * P]
                        for oh in range(n_oh):
                            nc.tensor.matmul(
                                ps[bc][oh],
                                lhsT,
                                wt[:, oh * 512 : (oh + 1) * 512],
                                start=False,
                                stop=stop,
                            )
            # deferred bank-4-7 tail (stag variant)
            for chd_, wt_, stop_ in deferred:
                for bc in (2, 3):
                    lhsT = chd_[:, bc * P : (bc + 1) * P]
                    for oh in range(n_oh):
                        nc.tensor.matmul(
                            ps[bc][oh], lhsT,
                            wt_[:, oh * 512 : (oh + 1) * 512],
                            start=False, stop=stop_,
                        )
            # Drain this half's PSUM to SBUF and then HBM. Copies alternate
            # between DVE and ACT to halve the bank-free latency.
            if variant == "nodrain":
                continue
            for bc in range(NBC):
                for oh in range(n_oh):
                    ot = opool.tile([P, 512], f32, tag="ot")
                    if (bc * NOH + oh) % 2 == 0:
                        nc.vector.tensor_copy(ot[:], ps[bc][oh])
                    else:
                        nc.scalar.copy(ot[:], ps[bc][oh])
                    r0 = h * HALF + bc * P
                    nc.sync.dma_start(
                        out[r0 : r0 + P, oh * 512 : (oh + 1) * 512], ot[:]
                    )


def _emit_body_pp(nc, tc, xt, w, out, ones, wbias,
                  wpool, xpool, tanpool, rpool, chpool, opool, pspool,
                  f32, bf16, mult, sub, Tanh):
    """Bank ping-pong: each half runs two passes over all k-blocks, one per
    bank group (bc 0-1 -> banks 0-3, bc 2-3 -> banks 4-7). A group's PSUM
    drain overlaps the other group's matmuls, removing the half-boundary
    serialization. Cheby tiles are computed in pass 0 and reused in pass 1;
    W tiles are re-streamed per pass (2x DMA, still under the PE floor)."""
    for h in range(N_HALF):
        ps = [
            [
                pspool.tile(
                    [P, 512], f32, tag=f"ps_{bc}_{oh}", name=f"ps_{bc}_{oh}"
                )
                for oh in range(NOH)
            ]
            for bc in range(NBC)
        ]
        chs = {}
        for p_ in range(2):
            bcs = (0, 1) if p_ == 0 else (2, 3)
            for bc in bcs:
                for oh in range(NOH):
                    nc.tensor.matmul(
                        ps[bc][oh],
                        ones,
                        wbias[:, oh * 512 : (oh + 1) * 512],
                        start=True,
                        stop=False,
                    )
            for i in range(NI):
                if p_ == 0:
                    xti = xpool.tile([P, HALF], f32, tag="x")
                    nc.sync.dma_start(
                        xti[:],
                        xt[i * P : (i + 1) * P, h * HALF : (h + 1) * HALF],
                    )
                    t = tanpool.tile([P, HALF], f32, tag="t")
                    nc.scalar.activation(t[:], xti[:], Tanh)
                    tm2, tm1 = None, t
                    for d in range(1, DEG + 1):
                        chd = chpool.tile([P, HALF], bf16, tag="ch",
                                          name=f"ch_{h}_{i}_{d}")
                        if d == 1:
                            nc.scalar.copy(chd[:], t[:])
                            cur = t
                        else:
                            pr = rpool.tile([P, HALF], f32, tag="rec")
                            nc.vector.scalar_tensor_tensor(
                                pr[:], tm1[:], 2.0, t[:], mult, mult
                            )
                            if d == 2:
                                cur = rpool.tile([P, HALF], f32, tag="rec")
                                nc.vector.tensor_scalar_sub(cur[:], pr[:], 1.0)
                                nc.scalar.copy(chd[:], cur[:])
                            elif d < DEG:
                                cur = rpool.tile([P, HALF], f32, tag="rec")
                                nc.vector.tensor_tensor(cur[:], pr[:], tm2[:], sub)
                                nc.scalar.copy(chd[:], cur[:])
                            else:
                                cur = None
                                nc.vector.tensor_tensor(chd[:], pr[:], tm2[:], sub)
                        tm2, tm1 = tm1, cur
                        chs[(i, d)] = chd
                for d in range(1, DEG + 1):
                    chd = chs[(i, d)]
                    wt = wpool.tile([P, OUT_F], bf16, tag="w")
                    nc.sync.dma_start(wt[:], w[d - 1, i * P : (i + 1) * P, :])
                    stop = i == NI - 1 and d == DEG
                    for bc in bcs:
                        lhsT = chd[:, bc * P : (bc + 1) * P]
                        for oh in range(NOH):
                            nc.tensor.matmul(
                                ps[bc][oh],
                                lhsT,
                                wt[:, oh * 512 : (oh + 1) * 512],
                                start=False,
                                stop=stop,
                            )
            # drain this bank group; overlaps the other group's compute
            for bc in bcs:
                for oh in range(NOH):
                    ot = opool.tile([P, 512], f32, tag="ot")
                    if (bc * NOH + oh) % 2 == 0:
                        nc.vector.tensor_copy(ot[:], ps[bc][oh])
                    else:
                        nc.scalar.copy(ot[:], ps[bc][oh])
                    r0 = h * HALF + bc * P
                    nc.sync.dma_start(
                        out[r0 : r0 + P, oh * 512 : (oh + 1) * 512], ot[:]
                    )


def _get_nc(loop_r=None, variant=""):
    key = (loop_r, variant)
    if key not in _CACHED_NC:
        _CACHED_NC[key] = _build_bass(loop_r, variant)
    return _CACHED_NC[key]


def _prep_inputs(x, coefficients):
    bf16 = ml_dtypes.bfloat16
    x = np.asarray(x, dtype=np.float32)
    coef = np.asarray(coefficients, dtype=np.float32)
    # (d, i, o) bf16 for d = 1..DEG
    w_all = np.ascontiguousarray(coef.transpose(2, 0, 1)[1 : DEG + 1]).astype(bf16)
    # d=0 term folded over i into a single 128-row contraction block
    # (ones-matmul trick used by the v0/pp/stag variants)
    wb_arr = np.ascontiguousarray(
        coef[:, :, 0].reshape(NI, P, OUT_F).sum(axis=0)
    ).astype(bf16)
    # d=0 term as a full-precision row, replicated across partitions and
    # added during the drain (s2 variants)
    br_row = coef[:, :, 0].sum(axis=0).astype(np.float32)
    br_arr = np.ascontiguousarray(
        np.broadcast_to(br_row[None, :], (P, OUT_F))
    ).astype(np.float32)
    in_maps = []
    for c in range(N_CORES):
        xc = x[c * B_CORE : (c + 1) * B_CORE, :]
        in_maps.append(
            {
                "xt": np.ascontiguousarray(xc.T),
                "w": w_all,
                "wb": wb_arr,
                "br": br_arr,
            }
        )
    return in_maps


def run(x, coefficients, trace=False, tmpdir=None, variant=""):
    """Run on hardware; returns (out, BassKernelResults)."""
    from concourse.bass_utils import run_bass_kernel_spmd

    nc = _get_nc(None, variant)
    in_maps = _prep_inputs(x, coefficients)
    res = run_bass_kernel_spmd(
        nc,
        in_maps,
        core_ids=list(range(N_CORES)),
        trace=trace,
        tmpdir=tmpdir,
    )
    out = np.concatenate([r["out"] for r in res.results], axis=0)
    return np.ascontiguousarray(out, dtype=np.float32), res


def kernel(x, coefficients):
    out, _ = run(x, coefficients, trace=False)
    return out


# revision 12
# speedup vs baseline: 1.0204x; 1.0175x over previous
"""ChebyKAN linear layer on 8 Trainium2 NeuronCores.

Computation: out[b,o] = sum_{i,d} T_d(tanh(x[b,i])) * coef[i,o,d]
  == sum_d T_d(tanh(x)) @ C_d   (8 accumulated 8192x1024x1024 matmuls
     for d=1..8; the d=0 term sum_i C_0[i,o] is a batch-independent row
     added during the PSUM drain)

Strategy:
  - Data-parallel over batch: core c handles rows [c*1024, (c+1)*1024).
  - Host pre-transposes each core's x slice to (in_features, batch) layout so
    the contraction dim (i) lands on SBUF partitions, and repacks the
    coefficients to (d, i, o) bf16.
  - On-chip: ACT computes tanh in fp32, DVE runs the Chebyshev recursion
    T_d = 2 t T_{d-1} - T_{d-2} in fp32 (scalar_tensor_tensor fuses the
    2*t*T_{d-1} product into one op), ACT casts each T_d to bf16, and PE
    accumulates the 8 degree-matmuls (d=1..8) in fp32 PSUM.
  - Per core the 1024-row batch is processed in two 512-column halves; each
    half keeps its full output (4 b-chunks x 2 o-halves) resident in all
    8 PSUM banks while 64 k-blocks accumulate into it.
  - "S-lag" schedule (default): within a half, the matmuls for batch chunks
    bc2/bc3 lag bc0/bc1 by S k-units. Each half therefore ends with a
    bc23-only tail and begins with a bc01-only head, so the PSUM drains of
    one bank group always overlap the other group's matmuls -- including
    across the half/iteration seam. This removes the ~40us of drain
    serialization the naive schedule pays per call.
  - Ldweights dedup: Tile emits one Ldweights per matmul; the second
    (o-half) matmul on the same stationary reloads it redundantly at ~53ns
    of serial PE time each. A post-compile pass removes exact duplicates,
    transferring any semaphore waits/updates onto the next instruction.

Numerics (validated on HW): rel l2 error vs fp32 reference ~2e-3.

Performance measured via on-device For_i loop slope (the axon tunnel's
~80 ms RPC overhead hides the kernel and NTFF profiling is unavailable
through it). Model: 1040 matmuls x ~210-227ns + ~520 ldweights x 53ns
+ seam slop.
"""

import numpy as np
import ml_dtypes

BATCH = 8192
IN_F = 1024
OUT_F = 1024
DEG = 8  # degree; DEG+1 coefficients per (i,o)
N_CORES = 8
B_CORE = BATCH // N_CORES  # 1024
P = 128
HALF = 512  # batch columns processed per PSUM-resident output block
NI = IN_F // P  # 8 contraction tiles
NBC = HALF // P  # 4 b-chunks per half
NOH = OUT_F // 512  # 2 output halves of 512
N_HALF = B_CORE // HALF  # 2
NU = NI * DEG  # 64 k-units per half

_CACHED_NC = {}


def _build_bass(loop_r=None, variant=""):
    """Build the Bass program. loop_r wraps the whole compute in a hardware
    For loop of loop_r iterations (benchmark-only; slope over loop_r gives
    per-iteration HW time since the axon RPC overhead is per-call)."""
    import contextlib

    import concourse.mybir as mybir
    import concourse.tile as tile
    from concourse import bacc

    f32 = mybir.dt.float32
    bf16 = mybir.dt.bfloat16
    mult = mybir.AluOpType.mult
    sub = mybir.AluOpType.subtract
    add = mybir.AluOpType.add
    Tanh = mybir.ActivationFunctionType.Tanh

    import json as _json

    def _dedup_ldweights(b):
        """Remove back-to-back InstLdweights that reload the identical
        stationary operand (the PE array still holds it). Tile emits one
        Ldweights per matmul, so a weight reused by consecutive matmuls is
        loaded twice; each redundant load costs ~53 ns of serial PE time.
        Semaphore waits/updates on a removed duplicate are transferred to
        the next kept instruction in the same queue (executes later in the
        same in-order stream, so ordering is preserved)."""
        n_removed = 0
        for fn in b.m.functions:
            for blk in fn.blocks:
                # per-engine state: blocks interleave all engines'
                # instructions, and only same-engine (PE) instructions can
                # disturb the loaded stationary or receive transferred sync
                last_key = {}
                pend = {}
                keep = []
                for inst in blk.instructions:
                    eng = getattr(inst, "engine", None)
                    if isinstance(inst, mybir.InstLdweights):
                        d = _json.loads(
                            mybir.instruction_to_pretty_json_string(inst)
                        )
                        key = _json.dumps(
                            [
                                d.get("ins"),
                                d.get("perf_mode"),
                                d.get("is_transpose"),
                                d.get("tile_position"),
                                d.get("tile_size"),
                            ],
                            sort_keys=True,
                        )
                        if key == last_key.get(eng):
                            si = inst.sync_info
                            if si is not None and (si.on_wait or si.on_update):
                                pw, pu = pend.setdefault(eng, ([], []))
                                pw.extend(list(si.on_wait or []))
                                pu.extend(list(si.on_update or []))
                            n_removed += 1
                            continue
                        last_key[eng] = key
                    elif isinstance(
                        inst, (mybir.InstMatmult, mybir.InstEventSemaphore)
                    ):
                        pass  # does not disturb loaded weights
                    else:
                        last_key.pop(eng, None)
                    if eng in pend:
                        pw, pu = pend.pop(eng)
                        si = inst.sync_info
                        if si is None:
                            raise RuntimeError(
                                "dedup: next inst has no sync_info to merge"
                            )
                        si.on_wait = list(si.on_wait or []) + pw
                        si.on_update = list(si.on_update or []) + pu
                    keep.append(inst)
                assert not pend, "dedup: dangling sync at block end"
                blk.instructions[:] = keep

    class _Bacc(bacc.Bacc):
        def compile(self):
            super().compile()
            _dedup_ldweights(self)

    nc = _Bacc(name="chebykan")
    xt = nc.dram_tensor("xt", (IN_F, B_CORE), f32, kind="ExternalInput")
    w = nc.dram_tensor("w", (DEG, IN_F, OUT_F), bf16, kind="ExternalInput")
    wb = nc.dram_tensor("wb", (P, OUT_F), bf16, kind="ExternalInput")
    br = nc.dram_tensor("br", (P, OUT_F), f32, kind="ExternalInput")
    out = nc.dram_tensor("out", (B_CORE, OUT_F), f32, kind="ExternalOutput")

    # Default schedule: "stag" (tail stagger) — empirically the fastest on
    # HW. The s2 S-lag schedule measured slower (drain overlap gains were
    # offset by DVE head-of-line blocking at the seams); kept for reference.
    is_s2 = variant.startswith("s2")
    if variant == "":
        variant = "stag"

    with (
        tile.TileContext(nc) as tc,
        tc.tile_pool(name="wpool", bufs=14) as wpool,
        tc.tile_pool(name="xpool", bufs=8) as xpool,
        tc.tile_pool(name="tanh", bufs=3) as tanpool,
        tc.tile_pool(name="rec", bufs=6) as rpool,
        tc.tile_pool(name="ch", bufs=80 if variant == "pp" else 20) as chpool,
        tc.tile_pool(name="const", bufs=1) as cpool,
        tc.tile_pool(name="outp", bufs=8) as opool,
        tc.tile_pool(name="psum", bufs=1, space="PSUM") as pspool,
    ):
        if is_s2:
            biasrow = cpool.tile([P, OUT_F], f32)
            nc.sync.dma_start(biasrow[:], br[:, :])
            ones = wbias = None
        else:
            ones = cpool.tile([P, P], bf16)
            nc.vector.memset(ones[:], 1.0)
            wbias = cpool.tile([P, OUT_F], bf16)
            nc.sync.dma_start(wbias[:], wb[:, :])
            biasrow = None

        loop_cm = (
            tc.For_i(
                0,
                loop_r,
                1,
                hint_engines=(mybir.EngineType.PE, mybir.EngineType.SP),
            )
            if loop_r is not None
            else contextlib.nullcontext()
        )
        with loop_cm:
            if is_s2:
                S = 6
                if "_" in variant:
                    S = int(variant.split("_")[1])
                _emit_body_s2(nc, tc, xt, w, out, biasrow, ones, wbias,
                              wpool, xpool, tanpool, rpool, chpool, opool,
                              pspool, f32, bf16, mult, sub, add, Tanh, S)
            else:
                _emit_body(nc, tc, xt, w, out, ones, wbias,
                           wpool, xpool, tanpool, rpool, chpool, opool,
                           pspool, f32, bf16, mult, sub, Tanh, variant)
    nc.finalize()
    return nc


def _emit_body_s2(nc, tc, xt, w, out, biasrow, ones, wbias,
                  wpool, xpool, tanpool, rpool, chpool, opool, pspool,
                  f32, bf16, mult, sub, add, Tanh, S):
    """S-lag schedule: bc2/bc3 matmuls lag bc0/bc1 by S k-units so each
    bank group's PSUM drain overlaps the other group's matmuls. Drains are
    DVE tensor_tensor adds that fuse the d=0 bias row; the bc01 drains are
    emitted BEFORE the bc23 tail so the DVE (which runs ahead of the PE and
    has already finished this half's production) executes them during the
    tail instead of head-of-line-blocking the next half's production."""
    Tanh_ = Tanh
    for h in range(N_HALF):
        ps = [
            [
                pspool.tile(
                    [P, 512], f32, tag=f"ps_{bc}_{oh}", name=f"ps_{bc}_{oh}"
                )
                for oh in range(NOH)
            ]
            for bc in range(NBC)
        ]
        ch_tiles = {}
        wts = {}
        rec_state = {}

        def produce(u):
            """Emit cheby production (DMA/ACT/DVE) for unit u=(i,d)."""
            i, d = u // DEG, u % DEG + 1
            if d == 1:
                xti = xpool.tile([P, HALF], f32, tag="x")
                nc.sync.dma_start(
                    xti[:],
                    xt[i * P : (i + 1) * P, h * HALF : (h + 1) * HALF],
                )
                t = tanpool.tile([P, HALF], f32, tag="t")
                nc.scalar.activation(t[:], xti[:], Tanh_)
                rec_state[i] = (None, t, t)  # tm2, tm1, t
            tm2, tm1, t = rec_state[i]
            chd = chpool.tile([P, HALF], bf16, tag="ch")
            if d == 1:
                nc.scalar.copy(chd[:], t[:])
                cur = t
            else:
                # pr = (T_{d-1} * 2) * t  (one fused DVE op)
                pr = rpool.tile([P, HALF], f32, tag="rec")
                nc.vector.scalar_tensor_tensor(
                    pr[:], tm1[:], 2.0, t[:], mult, mult
                )
                if d == 2:
                    cur = rpool.tile([P, HALF], f32, tag="rec")
                    nc.vector.tensor_scalar_sub(cur[:], pr[:], 1.0)
                    nc.scalar.copy(chd[:], cur[:])
                elif d < DEG:
                    cur = rpool.tile([P, HALF], f32, tag="rec")
                    nc.vector.tensor_tensor(cur[:], pr[:], tm2[:], sub)
                    nc.scalar.copy(chd[:], cur[:])
                else:
                    # final degree: write the bf16 tile directly
                    cur = None
                    nc.vector.tensor_tensor(chd[:], pr[:], tm2[:], sub)
            rec_state[i] = (tm1, cur, t)
            ch_tiles[u] = chd
            wt = wpool.tile([P, OUT_F], bf16, tag="w")
            nc.sync.dma_start(wt[:], w[d - 1, i * P : (i + 1) * P, :])
            wts[u] = wt

        def cell(u, bc):
            lhsT = ch_tiles[u][:, bc * P : (bc + 1) * P]
            wt = wts[u]
            for oh in range(NOH):
                nc.tensor.matmul(
                    ps[bc][oh],
                    lhsT,
                    wt[:, oh * 512 : (oh + 1) * 512],
                    start=(u == 0),
                    stop=(u == NU - 1),
                )

        def drain(bc, oh):
            # fused bias add: out_tile = psum + biasrow (the d=0 term)
            ot = opool.tile([P, 512], f32, tag="ot")
            nc.vector.tensor_tensor(
                ot[:], ps[bc][oh],
                biasrow[:, oh * 512 : (oh + 1) * 512], add,
            )
            r0 = h * HALF + bc * P
            nc.sync.dma_start(
                out[r0 : r0 + P, oh * 512 : (oh + 1) * 512], ot[:]
            )

        for u in range(NU):
            produce(u)
            cell(u, 0)
            cell(u, 1)
            if u >= S:
                cell(u - S, 2)
                cell(u - S, 3)
        # bc0/bc1 banks are complete (their stop fired at u=NU-1): drain
        # them now. The DVE reaches these after all production for this
        # half, the waits are already satisfied, and the PE's bc23 tail
        # below covers the copy latency.
        for bc in (0, 1):
            for oh in range(NOH):
                drain(bc, oh)
        for u in range(NU - S, NU):
            cell(u, 2)
            cell(u, 3)
        # bc2/bc3 drains: the next half's bc01-only head covers the bank
        # WAR; the DVE pays a short head-of-line wait here at the seam.
        for bc in (2, 3):
            for oh in range(NOH):
                drain(bc, oh)


def _emit_body(nc, tc, xt, w, out, ones, wbias,
               wpool, xpool, tanpool, rpool, chpool, opool, pspool,
               f32, bf16, mult, sub, Tanh, variant=""):
    if variant == "pp":
        _emit_body_pp(nc, tc, xt, w, out, ones, wbias,
                      wpool, xpool, tanpool, rpool, chpool, opool, pspool,
                      f32, bf16, mult, sub, Tanh)
        return
    n_oh = 1 if variant == "halfmm" else NOH
    for h in range(N_HALF):
            ps = [
                [
                    pspool.tile(
                        [P, 512], f32, tag=f"ps_{bc}_{oh}", name=f"ps_{bc}_{oh}"
                    )
                    for oh in range(n_oh)
                ]
                for bc in range(NBC)
            ]
            # Bias k-block: out += ones.T @ W_bias (covers the d=0 term).
            # start=True clears the PSUM banks.
            for bc in range(NBC):
                for oh in range(n_oh):
                    nc.tensor.matmul(
                        ps[bc][oh],
                        ones,
                        wbias[:, oh * 512 : (oh + 1) * 512],
                        start=True,
                        stop=False,
                    )
            deferred = []
            for i in range(NI):
                xti = xpool.tile([P, HALF], f32, tag="x")
                nc.sync.dma_start(
                    xti[:], xt[i * P : (i + 1) * P, h * HALF : (h + 1) * HALF]
                )
                t = tanpool.tile([P, HALF], f32, tag="t")
                nc.scalar.activation(t[:], xti[:], Tanh)

                tm2 = None  # T_{d-2} (fp32); None encodes T_0 == 1
                tm1 = t  # T_{d-1} (fp32)
                ch1 = None
                for d in range(1, DEG + 1):
                    last = d == DEG
                    if variant == "norec" and d > 1:
                        chd = ch1
                    else:
                        chd = chpool.tile([P, HALF], bf16, tag="ch")
                    if d == 1:
                        nc.scalar.copy(chd[:], t[:])
                        ch1 = chd
                        cur = t
                    elif variant == "norec":
                        cur = None
                    else:
                        # pr = (T_{d-1} * 2) * t  (one fused DVE op)
                        pr = rpool.tile([P, HALF], f32, tag="rec")
                        nc.vector.scalar_tensor_tensor(
                            pr[:], tm1[:], 2.0, t[:], mult, mult
                        )
                        if d == 2:
                            # T_2 = pr - 1
                            cur = rpool.tile([P, HALF], f32, tag="rec")
                            nc.vector.tensor_scalar_sub(cur[:], pr[:], 1.0)
                            nc.scalar.copy(chd[:], cur[:])
                        elif not last:
                            cur = rpool.tile([P, HALF], f32, tag="rec")
                            nc.vector.tensor_tensor(cur[:], pr[:], tm2[:], sub)
                            nc.scalar.copy(chd[:], cur[:])
                        else:
                            # final degree: write the bf16 tile directly
                            cur = None
                            nc.vector.tensor_tensor(chd[:], pr[:], tm2[:], sub)
                    tm2, tm1 = tm1, cur

                    if variant == "nodma":
                        if i == 0 and d == 1:
                            wt0 = wpool.tile([P, 1, OUT_F], bf16, tag="w")
                            nc.sync.dma_start(wt0[:, 0], w[0, 0:P, :])
                        wt = wt0[:, 0]
                    else:
                        wt = wpool.tile([P, OUT_F], bf16, tag="w")
                        nc.sync.dma_start(wt[:], w[d - 1, i * P : (i + 1) * P, :])
                    stop = i == NI - 1 and d == DEG
                    if variant == "stag" and i == NI - 1 and d >= 3:
                        # tail stagger: banks 0-3 finish their k-blocks
                        # before banks 4-7 start theirs, so the 0-3 drains
                        # overlap the 4-7 matmul tail
                        for bc in (0, 1):
                            lhsT = chd[:, bc * P : (bc + 1) * P]
                            for oh in range(n_oh):
                                nc.tensor.matmul(
                                    ps[bc][oh], lhsT,
                                    wt[:, oh * 512 : (oh + 1) * 512],
                                    start=False, stop=stop,
                                )
                        deferred.append((chd, wt, stop))
                        continue
                    for bc in range(NBC):
                        lhsT = chd[:, bc * P : (bc + 1) * P]
                        for oh in range(n_oh):
                            nc.tensor.matmul(
                                ps[bc][oh],
                                lhsT,
                                wt[:, oh * 512 : (oh + 1) * 512],
                                start=False,
                                stop=stop,
                            )
            # deferred bank-4-7 tail (stag variant)
            for chd_, wt_, stop_ in deferred:
                for bc in (2, 3):
                    lhsT = chd_[:, bc * P : (bc + 1) * P]
                    for oh in range(n_oh):
                        nc.tensor.matmul(
                            ps[bc][oh], lhsT,
                            wt_[:, oh * 512 : (oh + 1) * 512],
                            start=False, stop=stop_,
                        )
            # Drain this half's PSUM to SBUF and then HBM. Copies alternate
            # between DVE and ACT to halve the bank-free latency.
            if variant == "nodrain":
                continue
            for bc in range(NBC):
                for oh in range(n_oh):
                    ot = opool.tile([P, 512], f32, tag="ot")
                    if (bc * NOH + oh) % 2 == 0:
                        nc.vector.tensor_copy(ot[:], ps[bc][oh])
                    else:
                        nc.scalar.copy(ot[:], ps[bc][oh])
                    r0 = h * HALF + bc * P
                    nc.sync.dma_start(
                        out[r0 : r0 + P, oh * 512 : (oh + 1) * 512], ot[:]
                    )


def _emit_body_pp(nc, tc, xt, w, out, ones, wbias,
                  wpool, xpool, tanpool, rpool, chpool, opool, pspool,
                  f32, bf16, mult, sub, Tanh):
    """Bank ping-pong: each half runs two passes over all k-blocks, one per
    bank group (bc 0-1 -> banks 0-3, bc 2-3 -> banks 4-7). A group's PSUM
    drain overlaps the other group's matmuls, removing the half-boundary
    serialization. Cheby tiles are computed in pass 0 and reused in pass 1;
    W tiles are re-streamed per pass (2x DMA, still under the PE floor)."""
    for h in range(N_HALF):
        ps = [
            [
                pspool.tile(
                    [P, 512], f32, tag=f"ps_{bc}_{oh}", name=f"ps_{bc}_{oh}"
                )
                for oh in range(NOH)
            ]
            for bc in range(NBC)
        ]
        chs = {}
        for p_ in range(2):
            bcs = (0, 1) if p_ == 0 else (2, 3)
            for bc in bcs:
                for oh in range(NOH):
                    nc.tensor.matmul(
                        ps[bc][oh],
                        ones,
                        wbias[:, oh * 512 : (oh + 1) * 512],
                        start=True,
                        stop=False,
                    )
            for i in range(NI):
                if p_ == 0:
                    xti = xpool.tile([P, HALF], f32, tag="x")
                    nc.sync.dma_start(
                        xti[:],
                        xt[i * P : (i + 1) * P, h * HALF : (h + 1) * HALF],
                    )
                    t = tanpool.tile([P, HALF], f32, tag="t")
                    nc.scalar.activation(t[:], xti[:], Tanh)
                    tm2, tm1 = None, t
                    for d in range(1, DEG + 1):
                        chd = chpool.tile([P, HALF], bf16, tag="ch",
                                          name=f"ch_{h}_{i}_{d}")
                        if d == 1:
                            nc.scalar.copy(chd[:], t[:])
                            cur = t
                        else:
                            pr = rpool.tile([P, HALF], f32, tag="rec")
                            nc.vector.scalar_tensor_tensor(
                                pr[:], tm1[:], 2.0, t[:], mult, mult
                            )
                            if d == 2:
                                cur = rpool.tile([P, HALF], f32, tag="rec")
                                nc.vector.tensor_scalar_sub(cur[:], pr[:], 1.0)
                                nc.scalar.copy(chd[:], cur[:])
                            elif d < DEG:
                                cur = rpool.tile([P, HALF], f32, tag="rec")
                                nc.vector.tensor_tensor(cur[:], pr[:], tm2[:], sub)
                                nc.scalar.copy(chd[:], cur[:])
                            else:
                                cur = None
                                nc.vector.tensor_tensor(chd[:], pr[:], tm2[:], sub)
                        tm2, tm1 = tm1, cur
                        chs[(i, d)] = chd
                for d in range(1, DEG + 1):
                    chd = chs[(i, d)]
                    wt = wpool.tile([P, OUT_F], bf16, tag="w")
                    nc.sync.dma_start(wt[:], w[d - 1, i * P : (i + 1) * P, :])
                    stop = i == NI - 1 and d == DEG
                    for bc in bcs:
                        lhsT = chd[:, bc * P : (bc + 1) * P]
                        for oh in range(NOH):
                            nc.tensor.matmul(
                                ps[bc][oh],
                                lhsT,
                                wt[:, oh * 512 : (oh + 1) * 512],
                                start=False,
                                stop=stop,
                            )
            # drain this bank group; overlaps the other group's compute
            for bc in bcs:
                for oh in range(NOH):
                    ot = opool.tile([P, 512], f32, tag="ot")
                    if (bc * NOH + oh) % 2 == 0:
                        nc.vector.tensor_copy(ot[:], ps[bc][oh])
                    else:
                        nc.scalar.copy(ot[:], ps[bc][oh])
                    r0 = h * HALF + bc * P
                    nc.sync.dma_start(
                        out[r0 : r0 + P, oh * 512 : (oh + 1) * 512], ot[:]
                    )


def _get_nc(loop_r=None, variant=""):
    key = (loop_r, variant)
    if key not in _CACHED_NC:
        _CACHED_NC[key] = _build_bass(loop_r, variant)
    return _CACHED_NC[key]


def _prep_inputs(x, coefficients):
    bf16 = ml_dtypes.bfloat16
    x = np.asarray(x, dtype=np.float32)
    coef = np.asarray(coefficients, dtype=np.float32)
    # (d, i, o) bf16 for d = 1..DEG
    w_all = np.ascontiguousarray(coef.transpose(2, 0, 1)[1 : DEG + 1]).astype(bf16)
    # d=0 term folded over i into a single 128-row contraction block
    # (ones-matmul trick used by the v0/pp/stag variants)
    wb_arr = np.ascontiguousarray(
        coef[:, :, 0].reshape(NI, P, OUT_F).sum(axis=0)
    ).astype(bf16)
    # d=0 term as a full-precision row, replicated across partitions and
    # added during the drain (s2 variants)
    br_row = coef[:, :, 0].sum(axis=0).astype(np.float32)
    br_arr = np.ascontiguousarray(
        np.broadcast_to(br_row[None, :], (P, OUT_F))
    ).astype(np.float32)
    in_maps = []
    for c in range(N_CORES):
        xc = x[c * B_CORE : (c + 1) * B_CORE, :]
        in_maps.append(
            {
                "xt": np.ascontiguousarray(xc.T),
                "w": w_all,
                "wb": wb_arr,
                "br": br_arr,
            }
        )
    return in_maps


def run(x, coefficients, trace=False, tmpdir=None, variant=""):
    """Run on hardware; returns (out, BassKernelResults)."""
    from concourse.bass_utils import run_bass_kernel_spmd

    nc = _get_nc(None, variant)
    in_maps = _prep_inputs(x, coefficients)
    res = run_bass_kernel_spmd(
        nc,
        in_maps,
        core_ids=list(range(N_CORES)),
        trace=trace,
        tmpdir=tmpdir,
    )
    out = np.concatenate([r["out"] for r in res.results], axis=0)
    return np.ascontiguousarray(out, dtype=np.float32), res


def kernel(x, coefficients):
    out, _ = run(x, coefficients, trace=False)
    return out
